# revision 21
# baseline (speedup 1.0000x reference)
"""Exact KNN collision kernel for trn2 (8 NeuronCores).

nn[b,n] = argmin_m |vertices[b,n] - collider[b, cvi[m]]|^2, bit-exact fp32
scores with first-occurrence tie-breaking (matches the jax reference).

Per core (core c -> batch b=c//2, row-half h=c%2, 8192 rows, 64 tiles of 128):
  - PE: s = [v;1]^T @ [c; -|c|^2/2]  (K=4 fp32r matmul -> PSUM chunks)
    argmax_m s == argmin_m d2 exactly (d2 = -2s in fp32).
  - pass 1: running-max scan of s along candidates (tensor_tensor_scan max,
    split DVE/Pool, phase-chained via initial= carry) -> sc in SBUF.
  - r = max(r_dve_region, r_pool_region).
  - pass 2: counts of (sc < r) per chunk: ACT (Sign activation, scale=-1,
    bias=r, sum-accumulator) + DVE/Pool (scalar_tensor_tensor is_lt).
    Since sc is monotone per region, count == position of first occurrence;
    chunk counts compose: k = cnt1 + [cnt1==len1]*(cnt2 + [cnt2==len2]*(...)).
  - host maps dedup slot -> first position in collision_vertices.
"""
import os
import sys
import numpy as np

_BASS_PATH = "/opt/trn_rl_repo"
if _BASS_PATH not in sys.path:
    sys.path.insert(0, _BASS_PATH)

B, N, V, M = 4, 16384, 6890, 4096
NCORES = 8
ROWS = (B * N) // NCORES          # 8192 rows per core
NT = ROWS // 128                  # 64 row tiles

MM_DTYPE = os.environ.get("KNN_MM_DTYPE", "float32")

_PROGRAM_CACHE = {}


def _splits(U):
    """W = padded total columns (even halves); HALF per scan stream."""
    W = ((U + 3) // 4) * 4
    return W, W // 2


STRIDE = int(os.environ.get("KNN_STRIDE", "16"))   # count subsample stride


def _mm_chunks(a, b):
    # 512-aligned chunks: matmul output must not cross a PSUM bank boundary
    out = []
    s = a
    while s < b:
        e = min(s + 512, b)
        out.append((s, e))
        s = e
    assert all(x % 512 == 0 for x, y in out), out
    return out


def _build_program(U):
    import concourse.bacc as bacc
    import concourse.mybir as mybir
    import concourse.tile as tile

    f32 = mybir.dt.float32
    W, HALF = _splits(U)
    NS = HALF // STRIDE                              # count samples per tile

    nc = bacc.Bacc("TRN2", target_bir_lowering=False, debug=False, num_devices=NCORES)
    # host packs vc3 = [v rows (3) | candidate xyz (3)], candidates PERMUTED:
    # device cols [0,HALF) = slots 0,2,4,... ; [HALF,W) = slots 1,3,5,...
    # c2r = -|c|^2/2 replicated across 128 partitions, same permutation.
    vc3 = nc.dram_tensor("vc3", [3, ROWS + W], f32, kind="ExternalInput")
    c2r = nc.dram_tensor("c2r", [128, W], f32, kind="ExternalInput")
    out = nc.dram_tensor("idx", [NT // 8, 128, 8], f32, kind="ExternalOutput")

    NEG = -3.0e38
    mm = _mm_chunks(0, HALF)

    with tile.TileContext(nc) as tc:
        with (
            tc.tile_pool(name="const", bufs=1) as cpool,
            tc.tile_pool(name="sc", bufs=2) as scpool,
            tc.tile_pool(name="w", bufs=4) as wpool,
            tc.tile_pool(name="psum", bufs=1, space="PSUM") as ppool,
        ):
            vc_sb = cpool.tile([3, ROWS + W], f32)
            nc.sync.dma_start(vc_sb[:], vc3[:])
            c2_sb = cpool.tile([128, W], f32)
            nc.sync.dma_start(c2_sb[:], c2r[:])
            dummy = cpool.tile([128, 1], f32)
            nc.gpsimd.memset(dummy[:], 0.0)

            for t in range(NT):
                vT = vc_sb[:, t * 128:(t + 1) * 128]
                sc = scpool.tile([128, HALF], f32, tag="sc", name=f"sc{t}")
                sbE = scpool.tile([128, HALF], f32, tag="sbE", name=f"sbE{t}")
                sbO = scpool.tile([128, HALF], f32, tag="sbO", name=f"sbO{t}")

                psE = ppool.tile([128, HALF], f32, tag="psE", name=f"psE{t}")
                psO = ppool.tile([128, HALF], f32, tag="psO", name=f"psO{t}")
                for (ca, cb) in mm:
                    nc.tensor.matmul(psE[:, ca:cb], vT,
                                     vc_sb[:, ROWS + ca:ROWS + cb],
                                     start=True, stop=True)
                # s = fl(fl(dot) - c2h): bit-identical to the reference chain
                nc.vector.tensor_tensor(sbE[:], psE[:], c2_sb[:, 0:HALF],
                                        op=mybir.AluOpType.add)
                for (ca, cb) in mm:
                    nc.tensor.matmul(psO[:, ca:cb], vT,
                                     vc_sb[:, ROWS + HALF + ca:ROWS + HALF + cb],
                                     start=True, stop=True)
                nc.vector.tensor_tensor(sbO[:], psO[:], c2_sb[:, HALF:W],
                                        op=mybir.AluOpType.add)

                # adjacent-pair running max (order-preserving after permutation)
                nc.vector.tensor_tensor_scan(
                    sc[:], sbE[:], sbO[:], initial=NEG,
                    op0=mybir.AluOpType.max, op1=mybir.AluOpType.max)

                # coarse count on ACT: sign(r - sub) in {0,1}; accum = #(< r)
                ko = wpool.tile([128, 8], f32, tag="ko", name=f"ko{t // 8}", bufs=2) \
                    if t % 8 == 0 else ko  # noqa: F821
                trash = wpool.tile([128, NS], f32, tag="tr", name=f"tr{t}", bufs=2)
                nc.scalar.activation(
                    trash[:], sc[:, STRIDE - 1::STRIDE],
                    func=mybir.ActivationFunctionType.Sign,
                    bias=sc[:, HALF - 1:HALF], scale=-1.0,
                    accum_out=ko[:, t % 8:t % 8 + 1])
                if t % 8 == 7:
                    nc.sync.dma_start(out[t // 8], ko[:])
    nc.compile()
    return nc


def _get_program(U):
    if U not in _PROGRAM_CACHE:
        _PROGRAM_CACHE[U] = _build_program(U)
    return _PROGRAM_CACHE[U]


def kernel(vertices, collider, collision_vertices, _want_trace=False):
    from concourse.bass_utils import run_bass_kernel_spmd

    v = np.ascontiguousarray(np.asarray(vertices), dtype=np.float32)     # [B,N,3]
    c = np.ascontiguousarray(np.asarray(collider), dtype=np.float32)     # [B,V,3]
    cvi = np.asarray(collision_vertices).astype(np.int64)                # [M]

    # dedup candidates, keeping first-occurrence order (exact tie semantics)
    u, first_pos = np.unique(cvi, return_index=True)
    order = np.argsort(first_pos)
    u = u[order]
    first_pos = first_pos[order].astype(np.int32)
    U = len(u)
    W, HALF = _splits(U)

    cv = c[:, u, :]                                               # [B,U,3]
    import jax.numpy as _jnp
    c2 = np.asarray(_jnp.sum(_jnp.asarray(cv) * _jnp.asarray(cv), axis=-1))
    c2h = c2 * np.float32(-0.5)

    # permute candidates: device cols = [slots 0,2,4,... | slots 1,3,5,...]
    perm = np.concatenate([np.arange(0, W, 2), np.arange(1, W, 2)])
    cv_pad = np.zeros((B, 3, W), np.float32)
    cv_pad[:, :, :U] = cv.transpose(0, 2, 1)
    cv_pad = cv_pad[:, :, perm]
    c2h_pad = np.full((B, W), np.float32(-5e29), np.float32)
    c2h_pad[:, :U] = c2h
    c2h_pad = c2h_pad[:, perm]

    in_maps = []
    for core in range(NCORES):
        b = core // 2
        r0 = (core % 2) * ROWS
        v3 = v[b, r0:r0 + ROWS, :].T
        in_maps.append({
            "vc3": np.ascontiguousarray(
                np.concatenate([v3, cv_pad[b]], axis=1), dtype=np.float32),
            "c2r": np.ascontiguousarray(
                np.broadcast_to(c2h_pad[b][None, :], (128, W)), dtype=np.float32),
        })

    nc = _get_program(U)
    res = run_bass_kernel_spmd(nc, in_maps, core_ids=list(range(NCORES)))

    # --- host refinement: device returns the coarse pair-block index; the
    # exact argmin among its 2*STRIDE candidate columns is recomputed here
    # with the reference's own jax fp32 arithmetic (bit-matching tie cases).
    import jax
    import jax.numpy as jnp

    @jax.jit
    def _refine(vr, cvw, c2w, msk):
        d2 = c2w - 2.0 * jnp.einsum('rd,rkd->rk', vr, cvw)
        d2 = jnp.where(msk, d2, jnp.inf)
        return jnp.argmin(d2, axis=-1)

    nn = np.zeros((B, N), np.int32)
    for core in range(NCORES):
        b = core // 2
        r0 = (core % 2) * ROWS
        arr = res.results[core]["idx"].reshape(NT // 8, 128, 8)
        cblk = arr.transpose(0, 2, 1).reshape(-1)           # [tile, row] order
        cblk = np.clip(np.rint(cblk).astype(np.int64), 0, (HALF - 1) // STRIDE)
        slots = cblk[:, None] * (2 * STRIDE) + np.arange(2 * STRIDE)
        valid = slots < U
        sl = np.minimum(slots, U - 1)
        vr = v[b, r0:r0 + ROWS, :]                           # [ROWS, 3]
        win = np.asarray(_refine(jnp.asarray(vr), jnp.asarray(cv[b][sl]),
                                 jnp.asarray(c2[b][sl]), jnp.asarray(valid)))
        nn[b, r0:r0 + ROWS] = first_pos[sl[np.arange(ROWS), win]]
    batch_idx = np.broadcast_to(np.arange(B, dtype=np.int32)[:, None], nn.shape)
    outv = np.stack([batch_idx, nn], axis=-1).astype(np.int32)
    if _want_trace:
        return outv, (res, in_maps)
    return outv


# revision 22
# speedup vs baseline: 1.0044x; 1.0044x over previous
"""Exact KNN collision kernel for trn2 (8 NeuronCores).

nn[b,n] = argmin_m |vertices[b,n] - collider[b, cvi[m]]|^2, bit-exact fp32
scores with first-occurrence tie-breaking (matches the jax reference).

Per core (core c -> batch b=c//2, row-half h=c%2, 8192 rows, 64 tiles of 128):
  - PE: s = [v;1]^T @ [c; -|c|^2/2]  (K=4 fp32r matmul -> PSUM chunks)
    argmax_m s == argmin_m d2 exactly (d2 = -2s in fp32).
  - pass 1: running-max scan of s along candidates (tensor_tensor_scan max,
    split DVE/Pool, phase-chained via initial= carry) -> sc in SBUF.
  - r = max(r_dve_region, r_pool_region).
  - pass 2: counts of (sc < r) per chunk: ACT (Sign activation, scale=-1,
    bias=r, sum-accumulator) + DVE/Pool (scalar_tensor_tensor is_lt).
    Since sc is monotone per region, count == position of first occurrence;
    chunk counts compose: k = cnt1 + [cnt1==len1]*(cnt2 + [cnt2==len2]*(...)).
  - host maps dedup slot -> first position in collision_vertices.
"""
import os
import sys
import numpy as np

_BASS_PATH = "/opt/trn_rl_repo"
if _BASS_PATH not in sys.path:
    sys.path.insert(0, _BASS_PATH)

B, N, V, M = 4, 16384, 6890, 4096
NCORES = 8
ROWS = (B * N) // NCORES          # 8192 rows per core
NT = ROWS // 128                  # 64 row tiles

MM_DTYPE = os.environ.get("KNN_MM_DTYPE", "float32")

_PROGRAM_CACHE = {}


def _splits(U):
    """W = padded total columns (even halves); HALF per scan stream."""
    if U > 3584:
        W = 4096
    else:
        W = 3072 if U > 2048 else ((U + 1023) // 1024) * 1024
    return W, W // 2


STRIDE = int(os.environ.get("KNN_STRIDE", "16"))   # count subsample stride


def _mm_chunks(a, b):
    # 512-aligned chunks: matmul output must not cross a PSUM bank boundary
    out = []
    s = a
    while s < b:
        e = min(s + 512, b)
        out.append((s, e))
        s = e
    assert all(x % 512 == 0 for x, y in out), out
    return out


def _build_program(U):
    import concourse.bacc as bacc
    import concourse.mybir as mybir
    import concourse.tile as tile

    f32 = mybir.dt.float32
    W, HALF = _splits(U)
    NS = HALF // STRIDE                              # count samples per tile

    nc = bacc.Bacc("TRN2", target_bir_lowering=False, debug=False, num_devices=NCORES)
    # host packs vc3 = [v rows (3) | candidate xyz (3)], candidates PERMUTED:
    # device cols [0,HALF) = slots 0,2,4,... ; [HALF,W) = slots 1,3,5,...
    # c2r = -|c|^2/2 replicated across 128 partitions, same permutation.
    vc3 = nc.dram_tensor("vc3", [3, ROWS + W], f32, kind="ExternalInput")
    c2r = nc.dram_tensor("c2r", [128, W], f32, kind="ExternalInput")
    out = nc.dram_tensor("idx", [NT // 8, 128, 8], f32, kind="ExternalOutput")

    NEG = -3.0e38
    mm = _mm_chunks(0, HALF)

    with tile.TileContext(nc) as tc:
        with (
            tc.tile_pool(name="const", bufs=1) as cpool,
            tc.tile_pool(name="sc", bufs=2) as scpool,
            tc.tile_pool(name="w", bufs=4) as wpool,
            tc.tile_pool(name="psum", bufs=1, space="PSUM") as ppool,
        ):
            vc_sb = cpool.tile([3, ROWS + W], f32)
            nc.sync.dma_start(vc_sb[:], vc3[:])
            c2_sb = cpool.tile([128, W], f32)
            nc.sync.dma_start(c2_sb[:], c2r[:])
            dummy = cpool.tile([128, 1], f32)
            nc.gpsimd.memset(dummy[:], 0.0)

            for t in range(NT):
                vT = vc_sb[:, t * 128:(t + 1) * 128]
                sc = scpool.tile([128, HALF], f32, tag="sc", name=f"sc{t}")
                sbE = scpool.tile([128, HALF], f32, tag="sbE", name=f"sbE{t}")
                sbO = scpool.tile([128, HALF], f32, tag="sbO", name=f"sbO{t}")

                psE = ppool.tile([128, HALF], f32, tag="psE", name=f"psE{t}")
                psO = ppool.tile([128, HALF], f32, tag="psO", name=f"psO{t}")
                for (ca, cb) in mm:
                    nc.tensor.matmul(psE[:, ca:cb], vT,
                                     vc_sb[:, ROWS + ca:ROWS + cb],
                                     start=True, stop=True)
                # s = fl(fl(dot) - c2h): bit-identical to the reference chain
                nc.vector.tensor_tensor(sbE[:], psE[:], c2_sb[:, 0:HALF],
                                        op=mybir.AluOpType.add)
                for (ca, cb) in mm:
                    nc.tensor.matmul(psO[:, ca:cb], vT,
                                     vc_sb[:, ROWS + HALF + ca:ROWS + HALF + cb],
                                     start=True, stop=True)
                nc.vector.tensor_tensor(sbO[:], psO[:], c2_sb[:, HALF:W],
                                        op=mybir.AluOpType.add)

                # adjacent-pair running max (order-preserving after permutation)
                nc.vector.tensor_tensor_scan(
                    sc[:], sbE[:], sbO[:], initial=NEG,
                    op0=mybir.AluOpType.max, op1=mybir.AluOpType.max)

                # coarse count on ACT: sign(r - sub) in {0,1}; accum = #(< r)
                ko = wpool.tile([128, 8], f32, tag="ko", name=f"ko{t // 8}", bufs=2) \
                    if t % 8 == 0 else ko  # noqa: F821
                trash = wpool.tile([128, NS], f32, tag="tr", name=f"tr{t}", bufs=2)
                nc.scalar.activation(
                    trash[:], sc[:, STRIDE - 1::STRIDE],
                    func=mybir.ActivationFunctionType.Sign,
                    bias=sc[:, HALF - 1:HALF], scale=-1.0,
                    accum_out=ko[:, t % 8:t % 8 + 1])
                if t % 8 == 7:
                    nc.sync.dma_start(out[t // 8], ko[:])
    nc.compile()
    return nc


def _get_program(U):
    if U not in _PROGRAM_CACHE:
        _PROGRAM_CACHE[U] = _build_program(U)
    return _PROGRAM_CACHE[U]


def kernel(vertices, collider, collision_vertices, _want_trace=False):
    from concourse.bass_utils import run_bass_kernel_spmd

    v = np.ascontiguousarray(np.asarray(vertices), dtype=np.float32)     # [B,N,3]
    c = np.ascontiguousarray(np.asarray(collider), dtype=np.float32)     # [B,V,3]
    cvi = np.asarray(collision_vertices).astype(np.int64)                # [M]

    # dedup candidates, keeping first-occurrence order (exact tie semantics)
    u, first_pos = np.unique(cvi, return_index=True)
    order = np.argsort(first_pos)
    u = u[order]
    first_pos = first_pos[order].astype(np.int32)
    U = len(u)
    W, HALF = _splits(U)
    W_dev = W   # device covers slots [0, W); host refine checks the rest

    cv = c[:, u, :]                                               # [B,U,3]
    import jax.numpy as _jnp
    c2 = np.asarray(_jnp.sum(_jnp.asarray(cv) * _jnp.asarray(cv), axis=-1))
    c2h = c2 * np.float32(-0.5)

    # permute candidates: device cols = [slots 0,2,4,... | slots 1,3,5,...]
    perm = np.concatenate([np.arange(0, W, 2), np.arange(1, W, 2)])
    UW = min(U, W)
    cv_pad = np.zeros((B, 3, W), np.float32)
    cv_pad[:, :, :UW] = cv.transpose(0, 2, 1)[:, :, :UW]
    cv_pad = cv_pad[:, :, perm]
    c2h_pad = np.full((B, W), np.float32(-5e29), np.float32)
    c2h_pad[:, :UW] = c2h[:, :UW]
    c2h_pad = c2h_pad[:, perm]

    in_maps = []
    for core in range(NCORES):
        b = core // 2
        r0 = (core % 2) * ROWS
        v3 = v[b, r0:r0 + ROWS, :].T
        in_maps.append({
            "vc3": np.ascontiguousarray(
                np.concatenate([v3, cv_pad[b]], axis=1), dtype=np.float32),
            "c2r": np.ascontiguousarray(
                np.broadcast_to(c2h_pad[b][None, :], (128, W)), dtype=np.float32),
        })

    nc = _get_program(U)
    res = run_bass_kernel_spmd(nc, in_maps, core_ids=list(range(NCORES)))

    # --- host refinement: device returns the coarse pair-block index; the
    # exact argmin among its 2*STRIDE candidate columns is recomputed here
    # with the reference's own jax fp32 arithmetic (bit-matching tie cases).
    import jax
    import jax.numpy as jnp

    @jax.jit
    def _refine(vr, cvw, c2w, msk):
        d2 = c2w - 2.0 * jnp.einsum('rd,rkd->rk', vr, cvw)
        d2 = jnp.where(msk, d2, jnp.inf)
        return jnp.argmin(d2, axis=-1)

    nn = np.zeros((B, N), np.int32)
    for core in range(NCORES):
        b = core // 2
        r0 = (core % 2) * ROWS
        arr = res.results[core]["idx"].reshape(NT // 8, 128, 8)
        cblk = arr.transpose(0, 2, 1).reshape(-1)           # [tile, row] order
        cblk = np.clip(np.rint(cblk).astype(np.int64), 0, (HALF - 1) // STRIDE)
        n_ext = max(0, U - W_dev)
        slots = cblk[:, None] * (2 * STRIDE) + np.arange(2 * STRIDE)
        if n_ext:
            ext = np.broadcast_to(np.arange(W_dev, U), (ROWS, n_ext))
            slots = np.concatenate([slots, ext], 1)
        valid = slots < U
        sl = np.minimum(slots, U - 1)
        vr = v[b, r0:r0 + ROWS, :]                           # [ROWS, 3]
        win = np.asarray(_refine(jnp.asarray(vr), jnp.asarray(cv[b][sl]),
                                 jnp.asarray(c2[b][sl]), jnp.asarray(valid)))
        nn[b, r0:r0 + ROWS] = first_pos[sl[np.arange(ROWS), win]]
    batch_idx = np.broadcast_to(np.arange(B, dtype=np.int32)[:, None], nn.shape)
    outv = np.stack([batch_idx, nn], axis=-1).astype(np.int32)
    if _want_trace:
        return outv, (res, in_maps)
    return outv
